# revision 1
# baseline (speedup 1.0000x reference)
"""Trainium2 Bass kernel for nn_BasicBlock (conv3x3-BN-perelem_act-conv3x3-BN + act shortcut).

Data-parallel over batch: 32 images -> 4 per core x 8 cores.

Per-core layout: each 64x112x112 image is split into top/bottom 56-row halves,
mapped to SBUF partitions 0-63 (top, one per channel) and 64-127 (bottom), so
every elementwise op runs with all 128 lanes and the per-element activation
mask arrays need only a single copy.

Conv3x3 = 9 accumulating K=64 matmuls per 8-row output chunk, run as two
concurrent 64x64 array tiles (tile_position (0,0) for the top half and
(64,64) for the bottom half).

Per-element activation (codes 0..3 = relu/identity/tanh/sigmoid) is computed
as   act(y) = sigmoid(s1*y + s0) * w2 + F
with host-precomputed per-element arrays:
  s1 = {relu: 512, id: 0, tanh: 2, sigmoid: 1}
  s0 = {id: 40, else 0}            (sigmoid(40) == 1 -> identity passes y)
  CD = {tanh: 2, sigmoid: 1, else 0}  (w2 = y, overwritten by CD where CD != 0
                                       via one copy_predicated)
  F  = {tanh: -1, else 0}
BN is folded: scale via the ACT eviction pass (per-partition scale AP),
beta/mean folded into the host-side arrays (zero for this problem's fills).
"""

import os
import sys

sys.path.insert(0, "/opt/trn_rl_repo")

import numpy as np
import ml_dtypes
from contextlib import ExitStack

import concourse.bass as bass
import concourse.bacc as bacc
import concourse.tile as tile
import concourse.mybir as mybir
from concourse.bass_utils import run_bass_kernel_spmd

F16 = np.float16
MDT = mybir.dt.float16
EPS = 1e-5
KREL = 512.0   # sigmoid(KREL*y) ~ step(y) for the relu branch
SAT = 40.0     # sigmoid(40) == 1.0 for the identity branch

B, C, H, W = 32, 64, 112, 112
NCORES = 8
BPC = B // NCORES          # images per core
SEC = H // 2               # rows per half-section (56)
HP, WP = SEC + 2, W + 2    # padded section: 58 x 114
NU = SEC // 8              # 8-row elementwise units per half (7)

TAPS = [(ky, kx) for ky in (-1, 0, 1) for kx in (-1, 0, 1)]

LAST_RESULT = None  # BassKernelResults of the most recent kernel() call


def _split_halves(m):
    """[64, 112, X] -> [128, 56, X]: top rows on partitions 0-63, bottom on 64-127."""
    return np.concatenate([m[:, 0:SEC, :], m[:, SEC:H, :]], axis=0)


def _pad_split_image(img):
    """[64,112,112] fp -> [128, 58, 114] f16 padded split layout (1px halo)."""
    p = np.zeros((C, H + 2, W + 2), np.float32)
    p[:, 1:113, 1:113] = img
    top = p[:, 0:HP, :]
    bot = p[:, SEC:SEC + HP, :]
    return np.concatenate([top, bot], axis=0).astype(F16)


def _mask_arrays(codes, bn_b):
    """codes [C*H*W] int32 -> dict of split-layout [128,56,112] f16 arrays.
    bn_b: per-channel beta-fold (shape [C]) added where needed (F side only
    makes sense for the *final* combine; for the feature layer pass bn_b=0 and
    handle beta via the eviction bias path)."""
    c = codes.reshape(C, H, W)
    s1 = np.select([c == 0, c == 1, c == 2, c == 3], [KREL, 0.0, 2.0, 1.0]).astype(np.float32)
    s0 = np.where(c == 1, SAT, 0.0).astype(np.float32)
    cd = np.select([c == 2, c == 3], [2.0, 1.0], 0.0).astype(np.float32)
    f = np.where(c == 2, -1.0, 0.0).astype(np.float32) + bn_b[:, None, None]
    return {
        "s1": _split_halves(s1).astype(F16),
        "s0": _split_halves(s0).astype(F16),
        "cd": _split_halves(cd).astype(F16),
        "cm": _split_halves((cd != 0).astype(np.float32)).astype(np.uint8),
        "f": _split_halves(f).astype(F16),
    }


def _build_program():
    nc = bacc.Bacc("TRN2", target_bir_lowering=False, debug=False)

    xin = nc.dram_tensor("xin", [BPC, 128, HP, WP], MDT, kind="ExternalInput")
    w1d = nc.dram_tensor("w1", [9, 128, 64], MDT, kind="ExternalInput")
    w2d = nc.dram_tensor("w2", [9, 128, 64], MDT, kind="ExternalInput")
    a1d = nc.dram_tensor("a1", [128, 1], mybir.dt.float32, kind="ExternalInput")
    a2d = nc.dram_tensor("a2", [128, 1], mybir.dt.float32, kind="ExternalInput")
    mnames = ["s1f", "s0f", "cdf", "ff", "s1s", "s0s", "cds", "f2"]
    mdram = {
        k: nc.dram_tensor(k, [128, SEC, W], MDT, kind="ExternalInput") for k in mnames
    }
    for k in ("cmf", "cms"):  # uint8 predicate masks (CopyPredicated needs int dtype)
        mdram[k] = nc.dram_tensor(k, [128, SEC, W], mybir.dt.uint8, kind="ExternalInput")
    outd = nc.dram_tensor("out", [BPC, 128, SEC, W], MDT, kind="ExternalOutput")

    CP = mybir.ActivationFunctionType.Copy
    SG = mybir.ActivationFunctionType.Sigmoid

    with tile.TileContext(nc) as tc, ExitStack() as ctx:
        wp = ctx.enter_context(tc.tile_pool(name="w", bufs=1))
        mp = ctx.enter_context(tc.tile_pool(name="m", bufs=1))
        xp = ctx.enter_context(tc.tile_pool(name="x", bufs=1))
        hp = ctx.enter_context(tc.tile_pool(name="h", bufs=2))
        ep = ctx.enter_context(tc.tile_pool(name="e", bufs=2))
        op_ = ctx.enter_context(tc.tile_pool(name="o", bufs=2))
        pp = ctx.enter_context(tc.tile_pool(name="ps", bufs=4, space="PSUM"))

        w1t = wp.tile([128, 9, 64], MDT, tag="w1")
        w2t = wp.tile([128, 9, 64], MDT, tag="w2")
        for t in range(9):
            nc.sync.dma_start(w1t[:, t, :], w1d[t, :, :])
            nc.sync.dma_start(w2t[:, t, :], w2d[t, :, :])
        a1t = wp.tile([128, 1], mybir.dt.float32, tag="a1")
        a2t = wp.tile([128, 1], mybir.dt.float32, tag="a2")
        nc.sync.dma_start(a1t[:], a1d[:, :])
        nc.sync.dma_start(a2t[:], a2d[:, :])

        mt = {}
        for k in mnames:
            mt[k] = mp.tile([128, SEC, W], MDT, tag=k, name=k)
        for k in ("cmf", "cms"):
            mt[k] = mp.tile([128, SEC, W], mybir.dt.uint8, tag=k, name=k)
        obs = wp.tile([128, 2], MDT, tag="obs", name="obs")
        obu = wp.tile([128, 2], mybir.dt.uint8, tag="obu", name="obu")
        obg = wp.tile([128, 2], MDT, tag="obg", name="obg")
        # interleave DMA chunks and queue-observers by unit so the in-order
        # DVE only stalls on unit-0 chunks before image 0 starts (the rest
        # stream in behind compute)
        for u in range(NU):
            for k in mt:
                nc.sync.dma_start(mt[k][:, 8 * u:8 * u + 8, :],
                                  mdram[k][:, 8 * u:8 * u + 8, :])
            for k in mt:
                dst = obu if k in ("cmf", "cms") else obs
                nc.vector.tensor_add(dst[0:1, 0:1], mt[k][0:1, 8 * u, 0:1],
                                     mt[k][0:1, 8 * u, 0:1])
                if k in ("ff", "f2", "s0f", "s0s"):
                    nc.gpsimd.tensor_add(obg[0:1, 0:1], mt[k][0:1, 8 * u, 0:1],
                                         mt[k][0:1, 8 * u, 0:1])

        def conv_unit(src, wt, ps, r0):
            """9-tap conv into 2-bank psum tile ps[:, 0:8, 0:112] for output
            rows r0..r0+7 of each half; both halves concurrently."""
            for i in (0, 1):
                for t, (ky, kx) in enumerate(TAPS):
                    rs = r0 + 4 * i + 1 + ky
                    rhs_t = src[0:64, rs:rs + 4, kx + 1:kx + 113]
                    rhs_b = src[64:128, rs:rs + 4, kx + 1:kx + 113]
                    nc.tensor.matmul(
                        ps[0:64, 4 * i:4 * i + 4, 0:112], wt[0:64, t, :], rhs_t,
                        start=(t == 0), stop=(t == 8), tile_position=(0, 0),
                        skip_group_check=True,
                    )
                    nc.tensor.matmul(
                        ps[64:128, 4 * i:4 * i + 4, 0:112], wt[64:128, t, :], rhs_b,
                        start=(t == 0), stop=(t == 8), tile_position=(64, 64),
                        skip_group_check=True,
                    )

        for n in range(BPC):
            xt = xp.tile([128, HP, WP], MDT, tag="xt")
            nc.sync.dma_start(xt[:], xin[n, :, :, :])
            ht = hp.tile([128, HP, WP], MDT, tag="ht")
            if n < 2:
                # borders stay zero across reuses; interior is fully rewritten
                nc.gpsimd.memset(ht[:], 0.0)

            # ---- layer 1: conv1 -> BN1 -> per-element act -> ht
            for u in range(NU):
                r0 = 8 * u
                ps = pp.tile([128, 8, 128], mybir.dt.float32, tag="ps")
                conv_unit(xt, w1t, ps, r0)
                psv = ps[:, :, 0:112]
                y = ep.tile([128, 8, 112], MDT, tag="y", bufs=3)
                if u % 4 == 3:
                    nc.scalar.activation(y[:], psv, CP, scale=a1t[:])
                else:
                    nc.vector.tensor_scalar_mul(y[:], psv, a1t[:])
                xs = ep.tile([128, 8, 112], MDT, tag="xs")
                nc.vector.tensor_mul(xs[:], y[:], mt["s1f"][:, r0:r0 + 8, :])
                xs2 = ep.tile([128, 8, 112], MDT, tag="xs2")
                nc.vector.tensor_add(xs2[:], xs[:], mt["s0f"][:, r0:r0 + 8, :])
                sg = ep.tile([128, 8, 112], MDT, tag="sg", bufs=3)
                nc.scalar.activation(sg[:], xs2[:], SG)
                nc.vector.copy_predicated(
                    y[:], mt["cmf"][:, r0:r0 + 8, :], mt["cdf"][:, r0:r0 + 8, :])
                h2u = ep.tile([128, 8, 112], MDT, tag="h2u")
                nc.vector.tensor_mul(h2u[:], sg[:], y[:])
                hv = ht[:, r0 + 1:r0 + 9, 1:113]
                nc.gpsimd.tensor_add(hv, h2u[:], mt["ff"][:, r0:r0 + 8, :])

            # halo exchange between the two halves of ht (row 56 of the image
            # is the bottom half's first output row; row 55 is the top's last)
            nc.gpsimd.dma_start(ht[0:64, HP - 1, 1:113], ht[64:128, 1, 1:113])
            nc.gpsimd.dma_start(ht[64:128, 0, 1:113], ht[0:64, SEC, 1:113])

            # ---- layer 2: conv2 -> BN2 (+ shortcut act(x)) -> out
            for u in range(NU):
                r0 = 8 * u
                ps = pp.tile([128, 8, 128], mybir.dt.float32, tag="ps")
                conv_unit(ht, w2t, ps, r0)
                psv = ps[:, :, 0:112]
                y2 = ep.tile([128, 8, 112], MDT, tag="y2", bufs=3)
                if u % 4 == 3:
                    nc.scalar.activation(y2[:], psv, CP, scale=a2t[:])
                else:
                    nc.vector.tensor_scalar_mul(y2[:], psv, a2t[:])
                xu = ep.tile([128, 8, 112], MDT, tag="xu")
                nc.sync.dma_start(xu[:], xin[n, :, r0 + 1:r0 + 9, 1:113])
                nc.vector.tensor_add(obs[0:1, 1:2], xu[0:1, 0, 0:1], xu[0:1, 0, 0:1])
                xv = xu[:]
                t1 = ep.tile([128, 8, 112], MDT, tag="t1")
                nc.vector.tensor_mul(t1[:], xv, mt["s1s"][:, r0:r0 + 8, :])
                t2 = ep.tile([128, 8, 112], MDT, tag="t2")
                nc.vector.tensor_add(t2[:], t1[:], mt["s0s"][:, r0:r0 + 8, :])
                sg2 = ep.tile([128, 8, 112], MDT, tag="sg2", bufs=3)
                nc.scalar.activation(sg2[:], t2[:], SG)
                nc.vector.copy_predicated(
                    xv, mt["cms"][:, r0:r0 + 8, :], mt["cds"][:, r0:r0 + 8, :])
                z = ep.tile([128, 8, 112], MDT, tag="z")
                nc.vector.tensor_mul(z[:], sg2[:], xv)
                z2 = ep.tile([128, 8, 112], MDT, tag="z2")
                nc.gpsimd.tensor_add(z2[:], z[:], mt["f2"][:, r0:r0 + 8, :])
                o = op_.tile([128, 8, 112], MDT, tag="o", bufs=3)
                nc.gpsimd.tensor_add(o[:], y2[:], z2[:])
                nc.sync.dma_start(outd[n, :, r0:r0 + 8, :], o[:])

    nc.compile()
    return nc


def kernel(x, conv1_w, conv2_w, gamma1, beta1, mean1, var1,
           gamma2, beta2, mean2, var2, act_codes_feat, act_codes_sc):
    x = np.asarray(x, np.float32)
    a1 = (np.asarray(gamma1) / np.sqrt(np.asarray(var1) + EPS)).astype(np.float32)
    b1 = (np.asarray(beta1) - np.asarray(mean1) * a1).astype(np.float32)
    a2 = (np.asarray(gamma2) / np.sqrt(np.asarray(var2) + EPS)).astype(np.float32)
    b2 = (np.asarray(beta2) - np.asarray(mean2) * a2).astype(np.float32)

    mf = _mask_arrays(np.asarray(act_codes_feat), np.zeros(C, np.float32))
    ms = _mask_arrays(np.asarray(act_codes_sc), b2)

    # beta1 != 0 would need a bias in the L1 eviction; fold what we can and
    # fail loudly otherwise (the benchmark fills use beta=0, mean=0).
    assert np.allclose(b1, 0.0), "beta1/mean1 fold not implemented for nonzero values"

    w1h = np.zeros((9, 128, 64), F16)
    w2h = np.zeros((9, 128, 64), F16)
    for t, (ky, kx) in enumerate(TAPS):
        w1h[t, 0:64] = w1h[t, 64:128] = np.asarray(conv1_w)[:, :, ky + 1, kx + 1].T.astype(F16)
        w2h[t, 0:64] = w2h[t, 64:128] = np.asarray(conv2_w)[:, :, ky + 1, kx + 1].T.astype(F16)

    a1h = np.concatenate([a1, a1]).reshape(128, 1).astype(np.float32)
    a2h = np.concatenate([a2, a2]).reshape(128, 1).astype(np.float32)

    nc = _build_program()

    in_maps = []
    for core in range(NCORES):
        xs = np.stack([
            _pad_split_image(x[core * BPC + i]) for i in range(BPC)
        ])
        in_maps.append({
            "xin": xs,
            "w1": w1h, "w2": w2h, "a1": a1h, "a2": a2h,
            "s1f": mf["s1"], "s0f": mf["s0"], "cdf": mf["cd"], "ff": mf["f"],
            "s1s": ms["s1"], "s0s": ms["s0"], "cds": ms["cd"], "f2": ms["f"],
            "cmf": mf["cm"], "cms": ms["cm"],
        })

    res = run_bass_kernel_spmd(nc, in_maps, core_ids=list(range(NCORES)))
    global LAST_RESULT
    LAST_RESULT = res

    out = np.empty((B, C, H, W), np.float32)
    for core in range(NCORES):
        o = res.results[core]["out"]  # [BPC, 128, 56, 112] f16
        for i in range(BPC):
            img = np.concatenate([o[i, 0:64], o[i, 64:128]], axis=1)
            out[core * BPC + i] = img.astype(np.float32)
    return out


if __name__ == "__main__":
    rng = np.random.default_rng(0)
    inputs = {
        "x": rng.standard_normal((B, C, H, W), np.float32),
        "conv1_w": rng.standard_normal((C, C, 3, 3), np.float32) * 0.05,
        "conv2_w": rng.standard_normal((C, C, 3, 3), np.float32) * 0.05,
        "gamma1": np.ones(C, np.float32), "beta1": np.zeros(C, np.float32),
        "mean1": np.zeros(C, np.float32), "var1": np.ones(C, np.float32),
        "gamma2": np.ones(C, np.float32), "beta2": np.zeros(C, np.float32),
        "mean2": np.zeros(C, np.float32), "var2": np.ones(C, np.float32),
        "act_codes_feat": rng.integers(0, 4, C * H * W).astype(np.int32),
        "act_codes_sc": rng.integers(0, 4, C * H * W).astype(np.int32),
    }
    out = kernel(**inputs)
    print("out", out.shape, out.dtype, float(np.abs(out).max()))



# revision 2
# speedup vs baseline: 1.1094x; 1.1094x over previous
"""Trainium2 Bass kernel for nn_BasicBlock (conv3x3-BN-perelem_act-conv3x3-BN + act shortcut).

Data-parallel over batch: 32 images -> 4 per core x 8 cores.

Layout: each 64x112x112 image is split into top/bottom 56-row halves mapped to
SBUF partitions 0-63 (top, one per channel) and 64-127 (bottom), so every
elementwise op uses all 128 lanes.

Conv3x3: 9 accumulating matmuls per 4-row block with a BLOCK-DIAGONAL
128x128 f16 stationary [[W,0],[0,W]] — one matmul computes both halves
(PE cost is per output free element, so fusing the halves halves PE time
vs. two 64x64 tile matmuls). BN scale (gamma/sqrt(var+eps)) is folded into
the conv weights per output channel; BN1 bias rides the PSUM eviction
(ACT Identity with per-partition bias); BN2 bias is folded into the K map.

Per-element activation (codes 0..3 = relu/identity/tanh/sigmoid):
  act(y) = sigmoid(s1*y) * (b*y + a) + d
with host-precomputed per-element f16 arrays
  s1 = {relu: 512, id: 0, tanh: 2, sigmoid: 1}
  b  = {relu: 1, id: 2, else 0}        (id: sigmoid(0)=0.5 -> 0.5*2y = y)
  a  = {tanh: 2, sigmoid: 1, else 0}
  d  = {tanh: -1, else 0}
The d-terms never touch the device math: the feature-layer d propagates
through conv2 linearly, so K = conv2_bnfolded(d1map) + bn2_bias + d2map is
precomputed on host and added in the final combine.
"""

import sys

sys.path.insert(0, "/opt/trn_rl_repo")

import numpy as np
from contextlib import ExitStack

import concourse.bass as bass
import concourse.bacc as bacc
import concourse.tile as tile
import concourse.mybir as mybir
from concourse.bass_utils import run_bass_kernel_spmd

F16 = np.float16
MDT = mybir.dt.float16
EPS = 1e-5
KREL = 512.0   # sigmoid(KREL*y) ~ step(y) for the relu branch

B, C, H, W = 32, 64, 112, 112
NCORES = 8
BPC = B // NCORES          # images per core
SEC = H // 2               # rows per half-section (56)
HP, WP = SEC + 2, W + 2    # padded section: 58 x 114
NU = SEC // 8              # 8-row elementwise units per half (7)

TAPS = [(ky, kx) for ky in (-1, 0, 1) for kx in (-1, 0, 1)]
MASKS_F = ["s1f", "bf", "af"]
MASKS_S = ["s1s", "bs", "as_", "kmap"]

LAST_RESULT = None  # BassKernelResults of the most recent kernel() call


def _split_halves(m):
    """[64, 112, X] -> [128, 56, X]: top rows on partitions 0-63, bottom on 64-127."""
    return np.concatenate([m[:, 0:SEC, :], m[:, SEC:H, :]], axis=0)


def _pad_split_image(img):
    """[64,112,112] fp -> [128, 58, 114] f16 padded split layout (1px halo)."""
    p = np.zeros((C, H + 2, W + 2), np.float32)
    p[:, 1:113, 1:113] = img
    top = p[:, 0:HP, :]
    bot = p[:, SEC:SEC + HP, :]
    return np.concatenate([top, bot], axis=0).astype(F16)


def _mask_arrays(codes):
    """codes [C*H*W] int32 -> (s1, b, a, d) [64,112,112] f32 arrays."""
    c = codes.reshape(C, H, W)
    s1 = np.select([c == 0, c == 1, c == 2, c == 3], [KREL, 0.0, 2.0, 1.0]).astype(np.float32)
    b = np.select([c == 0, c == 1], [1.0, 2.0], 0.0).astype(np.float32)
    a = np.select([c == 2, c == 3], [2.0, 1.0], 0.0).astype(np.float32)
    d = np.where(c == 2, -1.0, 0.0).astype(np.float32)
    return s1, b, a, d


def _host_conv3x3(x, w):
    """x [C,H,W] f32, w [O,I,3,3] f32 -> [O,H,W] f32 (pad=1)."""
    xp = np.zeros((C, H + 2, W + 2), np.float32)
    xp[:, 1:H + 1, 1:W + 1] = x
    out = np.zeros((w.shape[0], H, W), np.float32)
    for ky in range(3):
        for kx in range(3):
            out += np.einsum(
                "ihw,oi->ohw",
                xp[:, ky:ky + H, kx:kx + W],
                w[:, :, ky, kx],
                optimize=True,
            )
    return out


def _build_program():
    nc = bacc.Bacc("TRN2", target_bir_lowering=False, debug=False)

    xin = nc.dram_tensor("xin", [BPC, 128, HP, WP], MDT, kind="ExternalInput")
    w1d = nc.dram_tensor("w1", [9, 128, 128], MDT, kind="ExternalInput")
    w2d = nc.dram_tensor("w2", [9, 128, 128], MDT, kind="ExternalInput")
    b1d = nc.dram_tensor("b1", [128, 1], mybir.dt.float32, kind="ExternalInput")
    mdram = {
        k: nc.dram_tensor(k, [128, SEC, W], MDT, kind="ExternalInput")
        for k in MASKS_F + MASKS_S
    }
    outd = nc.dram_tensor("out", [BPC, 128, SEC, W], MDT, kind="ExternalOutput")

    ID = mybir.ActivationFunctionType.Identity
    SG = mybir.ActivationFunctionType.Sigmoid

    with tile.TileContext(nc) as tc, ExitStack() as ctx:
        wp = ctx.enter_context(tc.tile_pool(name="w", bufs=1))
        mp = ctx.enter_context(tc.tile_pool(name="m", bufs=1))
        xp = ctx.enter_context(tc.tile_pool(name="x", bufs=2))
        hp = ctx.enter_context(tc.tile_pool(name="h", bufs=2))
        ep = ctx.enter_context(tc.tile_pool(name="e", bufs=2))
        pp = ctx.enter_context(tc.tile_pool(name="ps", bufs=2, space="PSUM"))

        w1t = wp.tile([128, 9, 128], MDT, tag="w1")
        w2t = wp.tile([128, 9, 128], MDT, tag="w2")
        for t in range(9):
            nc.sync.dma_start(w1t[:, t, :], w1d[t, :, :])
            nc.sync.dma_start(w2t[:, t, :], w2d[t, :, :])
        b1t = wp.tile([128, 1], mybir.dt.float32, tag="b1")
        nc.sync.dma_start(b1t[:], b1d[:, :])

        mt = {k: mp.tile([128, SEC, W], MDT, tag=k, name=k) for k in MASKS_F + MASKS_S}

        xts = {}
        hts = {}

        def load_x(n):
            xts[n] = xp.tile([128, HP, WP], MDT, tag="xt", name=f"xt{n}")
            nc.sync.dma_start(xts[n][:], xin[n, :, :, :])

        def load_masks(names):
            for u in range(NU):
                for k in names:
                    nc.sync.dma_start(mt[k][:, 8 * u:8 * u + 8, :],
                                      mdram[k][:, 8 * u:8 * u + 8, :])

        def conv_unit(src, wt, ps, r0):
            """9-tap block-diag conv into ps[:, 0:8, 0:112] for output rows
            r0..r0+7 of each half (both halves in one matmul)."""
            for i in (0, 1):
                for t, (ky, kx) in enumerate(TAPS):
                    rs = r0 + 4 * i + 1 + ky
                    nc.tensor.matmul(
                        ps[:, 4 * i:4 * i + 4, 0:112], wt[:, t, :],
                        src[:, rs:rs + 4, kx + 1:kx + 113],
                        start=(t == 0), stop=(t == 8),
                        skip_group_check=True,
                    )

        def l1_phase(n):
            load_x(n)
            if n == 0:
                load_masks(MASKS_F)
            elif n == 1:
                load_masks(MASKS_S)
            xt = xts[n]
            ht = hp.tile([128, HP, WP], MDT, tag="ht", name=f"ht{n}")
            hts[n] = ht
            if n < 2:
                # borders stay zero across reuses; interior is fully rewritten
                nc.gpsimd.memset(ht[:], 0.0)
            for u in range(NU):
                r0 = 8 * u
                ps = pp.tile([128, 8, 128], mybir.dt.float32, tag="ps1")
                conv_unit(xt, w1t, ps, r0)
                y16 = ep.tile([128, 8, 112], MDT, tag="y16")
                nc.scalar.activation(y16[:], ps[:, :, 0:112], ID, bias=b1t[:])
                m1 = ep.tile([128, 8, 112], MDT, tag="m1")
                nc.vector.tensor_mul(m1[:], y16[:], mt["s1f"][:, r0:r0 + 8, :])
                g = ep.tile([128, 8, 112], MDT, tag="g")
                nc.scalar.activation(g[:], m1[:], SG)
                m2 = ep.tile([128, 8, 112], MDT, tag="m2")
                nc.vector.tensor_mul(m2[:], y16[:], mt["bf"][:, r0:r0 + 8, :])
                nc.vector.tensor_add(m2[:], m2[:], mt["af"][:, r0:r0 + 8, :])
                nc.vector.tensor_mul(ht[:, r0 + 1:r0 + 9, 1:113], g[:], m2[:])
            # halo exchange between the two halves of ht
            nc.gpsimd.dma_start(ht[0:64, HP - 1, 1:113], ht[64:128, 1, 1:113])
            nc.gpsimd.dma_start(ht[64:128, 0, 1:113], ht[0:64, SEC, 1:113])

        def l2_phase(n):
            xt = xts[n]
            ht = hts[n]
            for u in range(NU):
                r0 = 8 * u
                ps = pp.tile([128, 8, 128], mybir.dt.float32, tag="ps2")
                conv_unit(ht, w2t, ps, r0)
                xi = xt[:, r0 + 1:r0 + 9, 1:113]
                t1 = ep.tile([128, 8, 112], MDT, tag="t1")
                nc.vector.tensor_mul(t1[:], xi, mt["s1s"][:, r0:r0 + 8, :])
                gs = ep.tile([128, 8, 112], MDT, tag="gs")
                nc.scalar.activation(gs[:], t1[:], SG)
                t2 = ep.tile([128, 8, 112], MDT, tag="t2")
                nc.gpsimd.tensor_mul(t2[:], xi, mt["bs"][:, r0:r0 + 8, :])
                nc.gpsimd.tensor_add(t2[:], t2[:], mt["as_"][:, r0:r0 + 8, :])
                sv = ep.tile([128, 8, 112], MDT, tag="sv")
                nc.vector.tensor_mul(sv[:], gs[:], t2[:])
                o1 = ep.tile([128, 8, 112], MDT, tag="o1", bufs=3)
                nc.vector.tensor_add(o1[:], ps[:, :, 0:112], sv[:])
                nc.vector.tensor_add(o1[:], o1[:], mt["kmap"][:, r0:r0 + 8, :])
                nc.sync.dma_start(outd[n, :, r0:r0 + 8, :], o1[:])

        for n in range(BPC):
            l1_phase(n)
            if n >= 1:
                l2_phase(n - 1)
        l2_phase(BPC - 1)

    nc.compile()
    return nc


def kernel(x, conv1_w, conv2_w, gamma1, beta1, mean1, var1,
           gamma2, beta2, mean2, var2, act_codes_feat, act_codes_sc):
    x = np.asarray(x, np.float32)
    a1 = (np.asarray(gamma1) / np.sqrt(np.asarray(var1) + EPS)).astype(np.float32)
    b1 = (np.asarray(beta1) - np.asarray(mean1) * a1).astype(np.float32)
    a2 = (np.asarray(gamma2) / np.sqrt(np.asarray(var2) + EPS)).astype(np.float32)
    b2 = (np.asarray(beta2) - np.asarray(mean2) * a2).astype(np.float32)

    s1f, bf, af, d1 = _mask_arrays(np.asarray(act_codes_feat))
    s1s, bs, as_, d2 = _mask_arrays(np.asarray(act_codes_sc))

    # fold BN scales into conv weights (per output channel)
    w1f = np.asarray(conv1_w, np.float32) * a1[:, None, None, None]
    w2f = np.asarray(conv2_w, np.float32) * a2[:, None, None, None]

    # K = conv2_bnfolded(d1map) + bn2 bias + d2map  (all constant)
    kmap = _host_conv3x3(d1, w2f) + b2[:, None, None] + d2

    # block-diagonal stationary weights [9, 128(k=in), 128(m=out)]
    w1h = np.zeros((9, 128, 128), F16)
    w2h = np.zeros((9, 128, 128), F16)
    for t, (ky, kx) in enumerate(TAPS):
        w1h[t, 0:64, 0:64] = w1h[t, 64:128, 64:128] = \
            w1f[:, :, ky + 1, kx + 1].T.astype(F16)
        w2h[t, 0:64, 0:64] = w2h[t, 64:128, 64:128] = \
            w2f[:, :, ky + 1, kx + 1].T.astype(F16)

    b1h = np.concatenate([b1, b1]).reshape(128, 1).astype(np.float32)

    nc = _build_program()

    marrs = {
        "s1f": s1f, "bf": bf, "af": af,
        "s1s": s1s, "bs": bs, "as_": as_, "kmap": kmap,
    }
    marrs = {k: _split_halves(v).astype(F16) for k, v in marrs.items()}

    in_maps = []
    for core in range(NCORES):
        xs = np.stack([
            _pad_split_image(x[core * BPC + i]) for i in range(BPC)
        ])
        im = {"xin": xs, "w1": w1h, "w2": w2h, "b1": b1h}
        im.update(marrs)
        in_maps.append(im)

    res = run_bass_kernel_spmd(nc, in_maps, core_ids=list(range(NCORES)))
    global LAST_RESULT
    LAST_RESULT = res

    out = np.empty((B, C, H, W), np.float32)
    for core in range(NCORES):
        o = res.results[core]["out"]  # [BPC, 128, 56, 112] f16
        for i in range(BPC):
            img = np.concatenate([o[i, 0:64], o[i, 64:128]], axis=1)
            out[core * BPC + i] = img.astype(np.float32)
    return out


if __name__ == "__main__":
    rng = np.random.default_rng(0)
    inputs = {
        "x": rng.standard_normal((B, C, H, W)).astype(np.float32),
        "conv1_w": (rng.standard_normal((C, C, 3, 3)) * 0.05).astype(np.float32),
        "conv2_w": (rng.standard_normal((C, C, 3, 3)) * 0.05).astype(np.float32),
        "gamma1": np.ones(C, np.float32), "beta1": np.zeros(C, np.float32),
        "mean1": np.zeros(C, np.float32), "var1": np.ones(C, np.float32),
        "gamma2": np.ones(C, np.float32), "beta2": np.zeros(C, np.float32),
        "mean2": np.zeros(C, np.float32), "var2": np.ones(C, np.float32),
        "act_codes_feat": rng.integers(0, 4, C * H * W).astype(np.int32),
        "act_codes_sc": rng.integers(0, 4, C * H * W).astype(np.int32),
    }
    out = kernel(**inputs)
    print("out", out.shape, out.dtype, float(np.abs(out).max()))
